# revision 13
# baseline (speedup 1.0000x reference)
"""Trainium2 Bass kernel for nn_BGNLLLoss (bivariate-Gaussian NLL loss).

Math (per element t,p):
    mux,muy,lsx,lsy,pc = params[t,p,:];  x,y = targets[t,p,:]
    sx=e^lsx, sy=e^lsy, c=tanh(pc), nr=1-c^2
    a=(x-mux)/sx, b=(y-muy)/sy
    nll = min( (a^2+b^2-2abc)/(2nr) + lsx+lsy + 0.5 ln(nr) + ln(2pi),
               -ln(1e-20) )
    loss[p] = sum_t nll[t,p]

tanh-free identity (single Exp table set; ln via exponent-bits trick).
With a' = a/sqrt2, b' = b/sqrt2:
  t4 = e^{-2 pc};  (a^2+b^2-2abc)/(2nr) = gvs^2 + b'^2
    with gv = a'*(1+t4) + b'*(t4-1),  gvs = gv * e^{pc}/2
  nll = min( gvs^2 + W2, CLAMP ),  W2 = lsx+lsy-pc-lvc+b'^2
    with lvc = ln(1+t4) - (ln2 + ln 2pi), computed from the bf16 bit
    pattern of (1+t4): ln(u) ~ (int16_bits(u)/2^7 - sigma)*ln2.

Layout/engine plan (per core; SPMD on 8 cores, persons sharded):
  Host de-interleaves the 7 channels [x,y,mux,muy,lsx,lsy,pc] into a
  person-major bf16 array [8 blocks, 128 persons, 7, 2048 frames]
  (halves HBM traffic; every SBUF operand is contiguous; adjacent
  channel pairs let one DVE op process two channels).
  Persons sit on partitions, so the frame-sum is a free accum_out on
  the final fused DVE op.
    ScalarE: isp=Exp(-[lsx|lsy]-ln2/2), t4, st2, lvc(bits), bsq=b'^2
    TensorE: W2 = I@lsx + I@lsy - I@pc - I@lvc + I@bsq
             (identity-weight matmuls accumulating in PSUM, 512/bank)
    VectorE: t4p1,t4m1 (TS 4x) | nxy=[x|y]-[mux|muy], abp=nxy*isp,
             avqn=abp*[t4p1|t4m1], gv=halves-sum, gvs=gv*st2 (TT 2x)
             minn(gvs^2 + W2, CLAMP) (custom, accum -> per-person sum)
    GpSimd : nothing (Pool shares the DVE SBUF port; keep it quiet)
"""

import math
from contextlib import ExitStack

import numpy as np
import ml_dtypes

import concourse.bass as bass
import concourse.bacc as bacc
import concourse.mybir as mybir
import concourse.tile as tile
from concourse import bass_utils
from concourse.dve_spec import Spec, Src0, Src1, C0, C1, lower, sq, minn, _has_src1
from concourse.dve_spec import AluOp
from concourse.dve_uop import DveOpSpec
import concourse.dve_ops as dve_ops

F32 = mybir.dt.float32
BF16 = mybir.dt.bfloat16
I16 = mybir.dt.int16
AF = mybir.ActivationFunctionType
ALU = mybir.AluOpType
BF16NP = ml_dtypes.bfloat16

T = 4096
P = 4096
N_CORES = 8
PC = P // N_CORES          # persons per core = 512
NG = PC // 128             # person groups of 128 = 4
FT = 2048                  # frames per block
NT = T // FT               # 2 frame chunks
NBLK = NG * NT             # 8 blocks
CW = 7 * FT                # per-partition row: 7 channels x FT = 14336

LOG2PI = math.log(2.0 * math.pi)
LN2 = math.log(2.0)
CADD = LN2 + LOG2PI                    # additive const inside the min
CLAMP = -math.log(1e-20)               # 46.0517...
B_ISP = -0.5 * LN2                     # exp bias: 1/(s? sqrt2)
B_ST2 = -LN2                           # exp bias: e^{pc}/2

# Fast-log constants (bf16 variant): for u = 2^e (1+f) >= 1, the bf16 bit
# pattern as int16 is bits = (e+127)*2^7 + f*2^7, so ln(u) ~=
# (bits/2^7 - (127 - c))*ln2 with the mantissa correction c = E[log2(1+f)-f]
# = 1.5 - 1/ln2. CADD is folded into sigma so lvc = ln(1+t4) - CADD.
LNK16 = LN2 / (1 << 7)
_C_MEAN = 1.5 - 1.0 / LN2
SIGMA16 = (127.0 - _C_MEAN + CADD / LN2) * (1 << 7)
B_LVC = -SIGMA16 * LNK16


# --------------------------------------------------------------------------
# Custom DVE op: nll = min(gvs^2 + W2 + C0, C1); accum_out = sum over frames
# --------------------------------------------------------------------------
def _register_dve_op(name: str, spec: Spec, subdim: bool = False):
    if name in dve_ops._SUB_OPCODE_FOR_NAME:
        return next(op for op in dve_ops.OPS if op.name == name)
    shas = {}
    for ver in ("v3", "v4"):
        uops = lower(spec, ver=ver)
        shas[ver] = DveOpSpec(
            name=name, opcode=0, uops=uops, rd1_en=_has_src1(spec)
        ).sha(ver)
    op = dve_ops.DveOp(name, spec, subdim=subdim, uops_sha=shas)
    dve_ops.OPS.append(op)
    dve_ops._SUB_OPCODE_FOR_NAME[name] = (
        dve_ops._CUSTOM_DVE_ROW_BASE + len(dve_ops.OPS) - 1
    )
    dve_ops.CUSTOM_DVE_SPECS[name] = spec
    return op


NLLSUM = _register_dve_op(
    "NLLSUM_BGNLL",
    Spec(body=minn(sq(Src0) + Src1 + C0, C1), accum=AluOp.ADD),
)


# --------------------------------------------------------------------------
# Kernel body (per core; SPMD -- same program on all 8 cores)
# --------------------------------------------------------------------------
def _emit(ctx: ExitStack, tc: tile.TileContext, inp: bass.AP, ident: bass.AP,
          cbias: bass.AP, loss: bass.AP):
    nc = tc.nc

    iop = ctx.enter_context(tc.tile_pool(name="iop", bufs=3))
    sp = ctx.enter_context(tc.tile_pool(name="sp", bufs=2))
    tp = ctx.enter_context(tc.tile_pool(name="tp", bufs=1))
    single = ctx.enter_context(tc.tile_pool(name="single", bufs=1))
    pp = ctx.enter_context(tc.tile_pool(name="pp", bufs=2, space="PSUM"))

    part = single.tile([128, NBLK], F32)
    out_sb = single.tile([128, NG], F32)
    id_sb = single.tile([128, 256], BF16)
    cb_sb = single.tile([128, 3], F32)

    sh = [128, FT]
    sh2 = [128, 2 * FT]
    ctxs: dict[int, dict] = {}

    def stage_load(blk):
        io = iop.tile([128, CW], BF16, tag="io")
        nc.sync.dma_start(io[:], inp[blk * 128:(blk + 1) * 128, :])
        ctxs[blk] = {"io": io}

    def stage_front(blk):
        c = ctxs[blk]
        io = c["io"]
        lsxv = io[:, 4 * FT:5 * FT]
        lsyv = io[:, 5 * FT:6 * FT]
        pcv = io[:, 6 * FT:7 * FT]
        c.update(xyv=io[:, 0:2 * FT], muv=io[:, 2 * FT:4 * FT])

        isp = sp.tile(sh2, BF16, tag="isp")
        t4 = sp.tile(sh, BF16, tag="t4")
        st2 = sp.tile(sh, BF16, tag="st2")
        t4pm = sp.tile(sh2, BF16, tag="t4pm")
        lvc = sp.tile(sh, BF16, tag="lvc")
        nc.scalar.activation(isp[:], io[:, 4 * FT:6 * FT], AF.Exp,
                             scale=-1.0, bias=cb_sb[:, 0:1])
        nc.scalar.activation(t4[:], pcv, AF.Exp, scale=-2.0)
        nc.scalar.activation(st2[:], pcv, AF.Exp, scale=1.0,
                             bias=cb_sb[:, 1:2])
        nc.vector.tensor_scalar_add(t4pm[:, 0:FT], t4[:], 1.0)
        nc.vector.tensor_scalar_add(t4pm[:, FT:2 * FT], t4[:], -1.0)
        nc.scalar.activation(lvc[:], t4pm[:, 0:FT].bitcast(I16), AF.Identity,
                             scale=LNK16, bias=cb_sb[:, 2:3])

        # W2 partial sums on the (otherwise idle) PE array:
        # w2 = lsx + lsy - pc - lvc  (+ b'^2 later, in stage_main)
        # One matmul per 512-col PSUM bank; term-outer keeps weight loads low.
        w2 = pp.tile(sh, F32, tag="w2")
        for w, src, st_, sp_ in ((id_sb[:, 0:128], lsxv, True, False),
                                 (id_sb[:, 0:128], lsyv, False, False),
                                 (id_sb[:, 128:256], pcv, False, False),
                                 (id_sb[:, 128:256], lvc[:], False, False)):
            for k in range(0, FT, 512):
                nc.tensor.matmul(w2[:, k:k + 512], w, src[:, k:k + 512],
                                 start=st_, stop=sp_)
        c.update(isp=isp, st2=st2, t4pm=t4pm, w2=w2)

    def stage_main(blk):
        c = ctxs[blk]
        nxy = tp.tile(sh2, BF16, tag="nxy")
        abp = tp.tile(sh2, BF16, tag="abp")
        avqn = tp.tile(sh2, BF16, tag="avqn")
        bsq = tp.tile(sh, BF16, tag="bsq")
        gv = tp.tile(sh, BF16, tag="gv")
        gvs = tp.tile(sh, BF16, tag="gvs")
        dead = tp.tile(sh, BF16, tag="dead")

        nc.vector.tensor_sub(nxy[:], c["xyv"], c["muv"])
        nc.vector.tensor_mul(abp[:], nxy[:], c["isp"][:])  # [a' | b']
        # b'^2 on ScalarE, accumulated into W2 by the PE array
        nc.scalar.activation(bsq[:], abp[:, FT:2 * FT], AF.Square)
        for k in range(0, FT, 512):
            nc.tensor.matmul(c["w2"][:, k:k + 512], id_sb[:, 0:128],
                             bsq[:, k:k + 512], start=False, stop=True)
        nc.vector.tensor_mul(avqn[:], abp[:], c["t4pm"][:])
        nc.vector.tensor_add(gv[:], avqn[:, 0:FT], avqn[:, FT:2 * FT])
        nc.vector.tensor_mul(gvs[:], gv[:], c["st2"][:])
        nc.vector._custom_dve(
            NLLSUM, out=dead[:], in0=gvs[:], in1=c["w2"][:],
            s0=0.0, s1=CLAMP, accum_out=part[:, blk:blk + 1],
        )
        del ctxs[blk]
        # finish a person group as soon as both its blocks are summed
        if blk % 2 == 1:
            g = blk // 2
            nc.vector.tensor_add(
                out_sb[:, g:g + 1], part[:, blk - 1:blk],
                part[:, blk:blk + 1],
            )

    # First input load goes out before everything else; the tiny const
    # loads ride behind it.
    stage_load(0)
    nc.sync.dma_start(id_sb[:], ident)
    nc.sync.dma_start(cb_sb[:], cbias)

    # Skewed emission (software pipelining): DMA for blk+2, then vector
    # consumers for blk, then front-end producers for blk+1 (so cross-engine
    # chains are emitted producer-first).
    for i in range(NBLK + 2):
        if 1 <= i < NBLK:
            stage_load(i)
        if 2 <= i and i - 2 < NBLK:
            stage_main(i - 2)
        if 1 <= i and i - 1 < NBLK:
            stage_front(i - 1)

    nc.sync.dma_start(loss, out_sb[:])


_CACHED_NC = None


def _build_program() -> bass.Bass:
    global _CACHED_NC
    if _CACHED_NC is not None:
        return _CACHED_NC
    nc = bacc.Bacc("TRN2", target_bir_lowering=False, debug=False,
                   enable_asserts=False)
    inp = nc.dram_tensor("inp", [NBLK * 128, CW], BF16,
                         kind="ExternalInput").ap()
    ident = nc.dram_tensor("ident", [128, 256], BF16,
                           kind="ExternalInput").ap()
    cbias = nc.dram_tensor("cbias", [128, 3], F32, kind="ExternalInput").ap()
    loss = nc.dram_tensor("loss", [128, NG], F32, kind="ExternalOutput").ap()
    with tile.TileContext(nc) as tc:
        with ExitStack() as ctx:
            _emit(ctx, tc, inp, ident, cbias, loss)
    nc.compile()
    _CACHED_NC = nc
    return nc


def _make_ident() -> np.ndarray:
    eye = np.eye(128, dtype=np.float32)
    return np.concatenate([eye, -eye], axis=1).astype(BF16NP)


def _make_cbias() -> np.ndarray:
    return np.tile(np.array([[B_ISP, B_ST2, B_LVC]], dtype=np.float32),
                   (128, 1))


def make_in_maps(targets: np.ndarray, params: np.ndarray):
    targets = np.asarray(targets, dtype=np.float32)
    params = np.asarray(params, dtype=np.float32)
    ident = _make_ident()
    cbias = _make_cbias()
    in_maps = []
    for ci in range(N_CORES):
        sl = slice(ci * PC, (ci + 1) * PC)
        chans = (
            targets[:, sl, 0], targets[:, sl, 1],
            params[:, sl, 0], params[:, sl, 1],
            params[:, sl, 2], params[:, sl, 3], params[:, sl, 4],
        )
        arr = np.empty((NG, NT, 128, 7, FT), dtype=BF16NP)
        for k, ch in enumerate(chans):
            v = ch.astype(BF16NP)                    # [T, PC]
            vv = v.reshape(NT, FT, NG, 128)          # [tc, t', g, p]
            arr[:, :, :, k, :] = vv.transpose(2, 0, 3, 1)
        in_maps.append({"inp": arr.reshape(NBLK * 128, CW), "ident": ident,
                        "cbias": cbias})
    return in_maps


def run_spmd(targets: np.ndarray, params: np.ndarray, trace: bool = False):
    nc = _build_program()
    in_maps = make_in_maps(targets, params)
    res = bass_utils.run_bass_kernel_spmd(
        nc, in_maps, core_ids=list(range(N_CORES)), trace=trace,
    )
    # results[i]["loss"][p, g] is person g*128+p of core i's slice
    loss = np.concatenate(
        [np.asarray(res.results[i]["loss"]).astype(np.float32).T.ravel()
         for i in range(N_CORES)]
    )
    return loss, res


def kernel(targets: np.ndarray, params: np.ndarray,
           peopleIDs: np.ndarray | None = None) -> np.ndarray:
    loss, _ = run_spmd(targets, params, trace=False)
    return loss
